# revision 45
# baseline (speedup 1.0000x reference)
"""Trainium2 Bass kernel for a pre-LN transformer decoder block.

Problem: x:[2,2048,1024] f32, causal mask, 16 heads, DFF=4096.
  out = x + Attn(LN1(x)); out = out + FFN(LN2(out))

Strategy v2 (8 NeuronCores, sequence-parallel with K/V AllGather):
  - Core c handles batch b=c//4; within the batch it OWNS 4 strided
    128-row tiles {i, i+4, i+8, i+12} (i = c%4) = 512 rows total.
  - LN1 + Q/K/V are computed only for the core's own 512 rows (no
    redundant K/V compute). K and V are then AllGather'ed within each
    batch's 4-core replica group through Internal-DRAM bounce buffers,
    giving every core the full 2048-token K/V.
  - Causal round-robin: local q-tile k (global tile i+4k) attends key
    tiles 0..4k+3 only -- a static, uniform loop across cores (62.5% of
    full attention work). The last 4 key tiles of each local q-tile get
    a data-driven mask (alive/diagonal/dead by core index) multiplied
    into exp() on the vector engine.
  - V is produced directly TRANSPOSED (keys on partitions) by swapping
    matmul operands (stationary=LN1 tile, moving=WvT), so no PE
    transposes are needed; softmax denominators come from ones columns
    interleaved in V.
  - LN gamma/beta are folded into the weights host-side; projection
    biases (W @ ln_b) are applied during PSUM evacuation.
  - Engine balance: Act does exp/relu/rstd only; evacuations go to
    DVE/Act split so exp overlaps the tensor engine.
  - Matmuls bf16 (f32 PSUM accumulate).
"""

import sys
import contextlib
import numpy as np

for _p in ("/opt/trn_rl_repo", "/root/.axon_site/_ro/trn_rl_repo"):
    if _p not in sys.path:
        sys.path.insert(0, _p)

import ml_dtypes  # noqa: E402
import concourse.bass as bass  # noqa: E402
import concourse.mybir as mybir  # noqa: E402
import concourse.tile as tile  # noqa: E402
from concourse import bacc  # noqa: E402
from concourse.bass_utils import run_bass_kernel_spmd  # noqa: E402
from concourse.hw_specs import get_activation_tables  # noqa: E402

P = 128
DH = 64
EPS = 1e-5
BF16 = mybir.dt.bfloat16
F32 = mybir.dt.float32
AF = mybir.ActivationFunctionType

_PROG_CACHE = {}


FULL_PHASES = frozenset({'lnq', 'qkv', 'cc', 'attn', 'mask', 'av', 'wo',
                         'ffn'})


def _build_program(S, D, H, DFF, TQ, n_iter=1, phases=FULL_PHASES):
    """One-core SPMD program. All cores run this identical program on
    different data; per-core causal structure lives in mask data."""
    J = D // P            # 8 feature tiles
    JF = DFF // P         # 32
    KC = S // P           # 16 key tiles
    NQT = TQ // P         # 4 local q-tiles
    G = S // TQ           # 4 cores per replica group
    VW = 2 * (DH + 1)     # v_sb cols per head-pair tile (130)
    assert TQ == 512 and H == 2 * J

    nc = bacc.Bacc(None, target_bir_lowering=False, num_devices=2 * G)

    # ---- I/O ----
    xqT = nc.dram_tensor("xqT", [D, TQ], F32, kind="ExternalInput")
    xqTb = nc.dram_tensor("xqTb", [D, TQ], BF16, kind="ExternalInput")
    maskT = nc.dram_tensor("maskT", [S, P], BF16, kind="ExternalInput")
    wqT = nc.dram_tensor("wqT", [D, D], BF16, kind="ExternalInput")
    wkT = nc.dram_tensor("wkT", [D, D], BF16, kind="ExternalInput")
    wvT = nc.dram_tensor("wvT", [D, D], BF16, kind="ExternalInput")
    woT = nc.dram_tensor("woT", [D, D], BF16, kind="ExternalInput")
    w1T = nc.dram_tensor("w1T", [D, DFF], BF16, kind="ExternalInput")
    w2T = nc.dram_tensor("w2T", [DFF, D], BF16, kind="ExternalInput")
    bq = nc.dram_tensor("bq", [D], F32, kind="ExternalInput")
    bk = nc.dram_tensor("bk", [D], F32, kind="ExternalInput")
    bvr = nc.dram_tensor("bvr", [D], BF16, kind="ExternalInput")
    b1 = nc.dram_tensor("b1", [DFF], F32, kind="ExternalInput")
    b2 = nc.dram_tensor("b2", [D], F32, kind="ExternalInput")
    outT = nc.dram_tensor("outT", [D, TQ], F32, kind="ExternalOutput")

    # ---- DRAM internals for the merged K+V collective ----
    # kvown cols: [0:J*TQ] = K by (m, t); [J*TQ:] = V^T by (kt, m, h2, d)
    KVW = 2 * J * TQ
    kvown = nc.dram_tensor("kvown", [P, KVW], BF16, kind="Internal")
    kvgat = nc.dram_tensor("kvgat", [G, P, KVW], BF16, kind="Internal")

    xqT_r = xqT.rearrange("(j p) t -> p j t", p=P)
    xqTb_r = xqTb.rearrange("(j p) t -> p j t", p=P)
    maskT_r = maskT.rearrange("(kc p) q -> p kc q", p=P)
    wqT_r = wqT.rearrange("(j p) e -> p j e", p=P)
    wkT_r = wkT.rearrange("(j p) e -> p j e", p=P)
    wvT_r = wvT.rearrange("(j p) e -> p j e", p=P)
    woT_r = woT.rearrange("(j p) e -> p j e", p=P)
    w1T_r = w1T.rearrange("(j p) f -> p j f", p=P)
    w2T_r = w2T.rearrange("(jf p) e -> p jf e", p=P)
    kvg_k = kvgat.rearrange("r p (w m t) -> r p w m t", w=2, m=J)
    kvg_v = kvgat.rearrange("r p (w kt m h d) -> r p w kt m h d",
                            w=2, kt=NQT, m=J, h=2)
    outT_r = outT.rearrange("(j p) q -> p j q", p=P)

    rgroups = [list(range(G)), list(range(G, 2 * G))]

    # NRT collectives cannot replay inside a hardware Fori loop (mesh
    # desync at iteration 2), so multi-iteration timing programs are
    # UNROLLED: straight-line sequential collectives are supported.
    # Collective-free ablations (no 'cc' phase) use a Fori loop.
    use_fori = n_iter > 1 and 'cc' not in phases and 'unroll' not in phases
    loop_cm = nc.Fori(0, n_iter) if use_fori else contextlib.nullcontext()
    with loop_cm, tile.TileContext(nc) as tc:
      for _it in range(1 if use_fori else n_iter):
        with (
            tc.tile_pool(name="const", bufs=1) as const,
            tc.tile_pool(name="persist", bufs=1) as persist,
            tc.tile_pool(name="scr", bufs=2) as scr,
            tc.tile_pool(name="scr_s", bufs=2) as scr_s,
        ):
            # Pre-load the one activation table that covers every function
            # we use (Ln/Exp/Identity/Copy/Relu); the automatic pass would
            # otherwise thrash exp_and_others <-> natural_log 5x/iter.
            if _it == 0:
                _tabs = list(get_activation_tables("gen3"))
                nc.scalar.add_instruction(mybir.InstLoadActFuncSet(
                    name=f"I-{nc.next_id()}", ins=[], outs=[],
                    act_func_set_id=_tabs.index("natural_log_exp_and_others")))

            # constants
            ones_col = const.tile([P, 1], BF16)
            nc.vector.memset(ones_col[:], 1.0)
            ones_row = const.tile([1, P], BF16)
            nc.vector.memset(ones_row[:], 1.0)
            eps_t = const.tile([1, 1], F32)
            nc.vector.memset(eps_t[:], EPS)
            bq_c = const.tile([P, J], F32)
            nc.sync.dma_start(bq_c[:], bq.rearrange("(j p) -> p j", p=P))
            bk_c = const.tile([P, J], F32)
            nc.sync.dma_start(bk_c[:], bk.rearrange("(j p) -> p j", p=P))
            bv_row = const.tile([1, D], BF16)
            nc.sync.dma_start(bv_row[:], bvr.rearrange("(o e) -> o e", o=1))
            b1_c = const.tile([P, JF], F32)
            nc.sync.dma_start(b1_c[:], b1.rearrange("(j p) -> p j", p=P))
            b2_c = const.tile([P, J], F32)
            nc.sync.dma_start(b2_c[:], b2.rearrange("(j p) -> p j", p=P))

            # persistent activations
            xq_sb = persist.tile([P, J, TQ], F32)
            nc.sync.dma_start(xq_sb[:], xqT_r)
            attn_sb = persist.tile([P, J, TQ], BF16)

            def ln_finalize(ps_x, ps_q, src_sb, out_sb, pool):
                """Given psum sums/sumsq over features: normalize src."""
                inv_d = 1.0 / D
                mu = scr_s.tile([1, TQ], F32, tag="mu", bufs=1)
                nc.scalar.mul(mu[:], ps_x[:1, :], inv_d)
                ex2 = scr_s.tile([1, TQ], F32, tag="ex2", bufs=1)
                nc.scalar.mul(ex2[:], ps_q[:1, :], inv_d)
                var = scr_s.tile([1, TQ], F32, tag="var", bufs=1)
                nc.vector.tensor_mul(var[:], mu[:], mu[:])
                nc.vector.tensor_sub(var[:], ex2[:], var[:])
                nc.scalar.activation(var[:], var[:], AF.Ln,
                                     bias=eps_t[:], scale=1.0)
                nc.scalar.activation(var[:], var[:], AF.Exp,
                                     bias=0.0, scale=-0.5)
                mub = scr_s.tile([1, TQ], BF16, tag="mub")
                nc.scalar.copy(mub[:], mu[:])
                rsb = scr_s.tile([1, TQ], BF16, tag="rsb")
                nc.scalar.copy(rsb[:], var[:])
                pmu = pool.tile([P, TQ], F32, tag="bcst")
                nc.tensor.matmul(pmu[:], ones_row[:], mub[:],
                                 start=True, stop=True)
                prs = pool.tile([P, TQ], F32, tag="bcst")
                nc.tensor.matmul(prs[:], ones_row[:], rsb[:],
                                 start=True, stop=True)
                for j in range(J):
                    t1 = scr.tile([P, TQ], F32, tag="t1")
                    nc.vector.tensor_sub(t1[:], src_sb[:, j, :], pmu[:])
                    nc.vector.tensor_mul(out_sb[:, j, :], t1[:], prs[:])

            def layer_norm(src_sb, out_sb, pool):
                """z = (x - mu) * rstd over J*P features; src/out bf16."""
                ps_x = pool.tile([P, TQ], F32, tag="stat")
                ps_q = pool.tile([P, TQ], F32, tag="stat")
                for j in range(J):
                    xb = src_sb[:, j, :]
                    sq = scr.tile([P, TQ], BF16, tag="sq")
                    nc.vector.tensor_mul(sq[:], xb, xb)
                    nc.tensor.matmul(ps_x[:1, :], ones_col[:], xb,
                                     start=(j == 0), stop=(j == J - 1))
                    nc.tensor.matmul(ps_q[:1, :], ones_col[:], sq[:],
                                     start=(j == 0), stop=(j == J - 1))
                ln_finalize(ps_x, ps_q, src_sb, out_sb, pool)

            # attention-data pool (lives through attention)
            with tc.tile_pool(name="attd", bufs=1) as attd:
                q_sb = attd.tile([P, J, TQ], BF16)
                k_sb = attd.tile([P, J, S], BF16)
                v_sb = attd.tile([P, KC, J, 2, DH + 1], BF16)
                mask_sb = attd.tile([P, KC, P], BF16)
                nc.sync.dma_start(mask_sb[:], maskT_r)

                # ---------- Phase A: LN1 + Q/K/V + collective ----------
                with tc.tile_pool(name="sA", bufs=1) as sA, \
                     tc.tile_pool(name="sAw", bufs=4) as sAw, \
                     tc.tile_pool(name="ps_mm", bufs=2, space="PSUM") as ps_mm, \
                     tc.tile_pool(name="ps_vt", bufs=1, space="PSUM") as ps_vt:
                    ln1_own = sA.tile([P, J, TQ], BF16)
                    xqb_sb = sA.tile([P, J, TQ], BF16)
                    if 'lnq' in phases:
                        nc.sync.dma_start(xqb_sb[:], xqTb_r)
                        layer_norm(xqb_sb, ln1_own, ps_mm)
                    # K projection (own rows) -> kv_stage[:, m*TQ:]
                    kv_stage = sA.tile([P, KVW], BF16)
                    for mm in range(J // 2 if 'qkv' in phases else 0):
                        ws = slice(2 * mm * P, (2 * mm + 2) * P)
                        wt = sAw.tile([P, J, 2 * P], BF16, tag="w8")
                        nc.sync.dma_start(wt[:], wkT_r[:, :, ws])
                        for m2 in range(2):
                            m = 2 * mm + m2
                            ps = ps_mm.tile([P, TQ], F32, tag="mm")
                            for j in range(J):
                                nc.tensor.matmul(
                                    ps[:], wt[:, j, m2 * P:(m2 + 1) * P],
                                    ln1_own[:, j, :],
                                    start=(j == 0), stop=(j == J - 1))
                            nc.scalar.activation(
                                kv_stage[:, m * TQ:(m + 1) * TQ], ps[:],
                                AF.Identity, bias=bk_c[:, m:m + 1], scale=1.0)
                    # V^T projection (own rows, keys on partitions) ->
                    # kv_stage[:, J*TQ + kt*D + m*P :]
                    wv_w = sA.tile([P, J, D], BF16)
                    if 'qkv' in phases:
                        nc.sync.dma_start(wv_w[:], wvT_r)
                    for kt in range(NQT if 'qkv' in phases else 0):
                        pv = ps_vt.tile([P, D], F32, tag="vt")
                        for j in range(J):
                            for eh in range(2):
                                es = slice(eh * TQ, (eh + 1) * TQ)
                                nc.tensor.matmul(
                                    pv[:, es], ln1_own[:, j, kt * P:(kt + 1) * P],
                                    wv_w[:, j, es], start=(j == 0), stop=False)
                        for eh in range(2):
                            es = slice(eh * TQ, (eh + 1) * TQ)
                            nc.tensor.matmul(pv[:, es], ones_row[:],
                                             bv_row[:, es],
                                             start=False, stop=True)
                        vb = J * TQ + kt * D
                        for m in range(J):
                            nc.vector.tensor_copy(
                                kv_stage[:, vb + m * P:vb + (m + 1) * P],
                                pv[:, m * P:(m + 1) * P])
                    if 'qkv' in phases:
                        nc.sync.dma_start(kvown[:], kv_stage[:])
                        if 'cc' in phases:
                            nc.gpsimd.collective_compute(
                                "AllGather", mybir.AluOpType.bypass,
                                replica_groups=rgroups,
                                ins=[kvown[:]], outs=[kvgat[:]])
                        # softmax-denominator ones columns
                        nc.vector.memset(v_sb[:, :, :, :, DH], 1.0)
                        # reload gathered K/V into SBUF (natural tile order)
                        for g in range(KC):
                            r, pos = g % G, g // G
                            nc.scalar.dma_start(
                                k_sb[:, :, g * P:(g + 1) * P],
                                kvg_k[r, :, 0, :, pos * P:(pos + 1) * P])
                            nc.sync.dma_start(
                                v_sb[:, g, :, :, 0:DH], kvg_v[r, :, 1, pos])
                    # Q projection (own rows)
                    for mm in range(J // 2 if 'lnq' in phases else 0):
                        ws = slice(2 * mm * P, (2 * mm + 2) * P)
                        wt = sAw.tile([P, J, 2 * P], BF16, tag="w8")
                        nc.sync.dma_start(wt[:], wqT_r[:, :, ws])
                        for m2 in range(2):
                            m = 2 * mm + m2
                            ps = ps_mm.tile([P, TQ], F32, tag="mm")
                            for j in range(J):
                                nc.tensor.matmul(
                                    ps[:], wt[:, j, m2 * P:(m2 + 1) * P],
                                    ln1_own[:, j, :],
                                    start=(j == 0), stop=(j == J - 1))
                            nc.scalar.activation(
                                q_sb[:, m, :], ps[:], AF.Identity,
                                bias=bq_c[:, m:m + 1], scale=1.0)

                # ---------- Phase B: attention ----------
                with tc.tile_pool(name="sBe", bufs=6) as sBe, \
                     tc.tile_pool(name="ps_sc", bufs=2, space="PSUM") as ps_sc, \
                     tc.tile_pool(name="ps_zn", bufs=1, space="PSUM") as ps_zn, \
                     tc.tile_pool(name="ps_av", bufs=3, space="PSUM") as ps_av:
                    do_av_phase = 'av' in phases
                    if 'attn' in phases and not do_av_phase:
                        nc.vector.memset(attn_sb[:], 0.0)  # ablation filler
                    for m in range(J if 'attn' in phases else 0):
                        if do_av_phase:
                            pavs = [ps_av.tile([P, TQ], F32, tag="av",
                                               name=f"pav{m}_{i}")
                                    for i in range(2)]
                        pend = []

                        def do_av(kc0, e0):
                            if not do_av_phase:
                                return
                            g0 = kc0 // G
                            q0 = g0 * P
                            for h2 in range(2):
                                vs = v_sb[:, kc0, m, h2, :]
                                nc.tensor.matmul(
                                    pavs[h2][:DH + 1, q0:],
                                    vs, e0[:, h2, q0:],
                                    start=(kc0 == 0), stop=(kc0 == KC - 1),
                                    skip_group_check=True)

                        for kc in range(KC):
                            g = kc // G
                            q0 = g * P
                            ps2 = ps_sc.tile([P, 2, TQ], F32, tag="sc")
                            nc.tensor.matmul(
                                ps2[:, 0, q0:],
                                k_sb[0:DH, m, kc * P:(kc + 1) * P],
                                q_sb[0:DH, m, q0:], start=True, stop=True,
                                tile_position=(0, 0))
                            nc.tensor.matmul(
                                ps2[:, 1, q0:],
                                k_sb[DH:P, m, kc * P:(kc + 1) * P],
                                q_sb[DH:P, m, q0:], start=True, stop=True,
                                tile_position=(DH, 0))
                            ex2 = sBe.tile([P, 2, TQ], BF16, tag="ex")
                            efun = AF.Identity if 'expid' in phases else AF.Exp
                            if 'exp2' in phases:
                                for h2 in range(2):
                                    nc.scalar.activation(
                                        ex2[:, h2, q0:], ps2[:, h2, q0:],
                                        efun, scale=1.0 / np.sqrt(DH))
                            else:
                                nc.scalar.activation(ex2[:, :, q0:],
                                                     ps2[:, :, q0:], efun,
                                                     scale=1.0 / np.sqrt(DH))
                            if 'mask' in phases:
                                nc.vector.tensor_mul(
                                    ex2[:, 0, q0:q0 + P], ex2[:, 0, q0:q0 + P],
                                    mask_sb[:, kc, :])
                                nc.vector.tensor_mul(
                                    ex2[:, 1, q0:q0 + P], ex2[:, 1, q0:q0 + P],
                                    mask_sb[:, kc, :])
                            pend.append((kc, ex2))
                            while len(pend) > 3:
                                kc0, e0 = pend.pop(0)
                                do_av(kc0, e0)
                        for kc0, e0 in pend:
                            do_av(kc0, e0)
                        for h2 in range(2 if do_av_phase else 0):
                            lo = h2 * DH
                            pav = pavs[h2]
                            zr = scr_s.tile([1, TQ], F32, tag="zr")
                            nc.vector.reciprocal(zr[:], pav[DH:DH + 1, :])
                            zrb = scr_s.tile([1, TQ], BF16, tag="zrb")
                            nc.vector.tensor_copy(zrb[:], zr[:])
                            ps_z = ps_zn.tile([P, TQ], F32, tag="zn")
                            nc.tensor.matmul(ps_z[:DH, :], ones_row[:, :DH],
                                             zrb[:], start=True, stop=True)
                            zb = scr_s.tile([DH, TQ], F32, tag="zb")
                            nc.vector.tensor_copy(zb[:], ps_z[:DH, :])
                            if h2 == 0:
                                # head0 rows align with attn_sb partitions:
                                # normalize writes attn_sb directly
                                nc.vector.tensor_mul(attn_sb[0:DH, m, :],
                                                     pav[:DH, :], zb[:])
                            else:
                                stg64 = scr_s.tile([DH, TQ], BF16,
                                                   tag="stg64")
                                nc.vector.tensor_mul(stg64[:], pav[:DH, :],
                                                     zb[:])
                                nc.sync.dma_start(attn_sb[lo:lo + DH, m, :],
                                                  stg64[:])

            # ---------- Phase C: Wo + residual, LN2, FFN ----------
            with tc.tile_pool(name="sC", bufs=1) as sC, \
                 tc.tile_pool(name="sCw", bufs=3) as sCw, \
                 tc.tile_pool(name="ps_mc", bufs=2, space="PSUM") as ps_mc:
                y_sb = sC.tile([P, J, TQ], F32)
                yb_sb = sC.tile([P, J, TQ], BF16)
                do_ln2 = 'ffn' in phases and 'wo' in phases
                if do_ln2:
                    ps_y = ps_mc.tile([P, TQ], F32, tag="lny")
                    ps_yq = ps_mc.tile([P, TQ], F32, tag="lny")
                for half in range(2 if 'wo' in phases else 0):
                    ws = slice(half * 4 * P, (half + 1) * 4 * P)
                    wt = sCw.tile([P, J, 4 * P], BF16, tag="w8o", bufs=2)
                    nc.sync.dma_start(wt[:], woT_r[:, :, ws])
                    for m4 in range(4):
                        m = half * 4 + m4
                        ps = ps_mc.tile([P, TQ], F32, tag="mm")
                        for j in range(J):
                            nc.tensor.matmul(
                                ps[:], wt[:, j, m4 * P:(m4 + 1) * P],
                                attn_sb[:, j, :],
                                start=(j == 0), stop=(j == J - 1))
                        nc.vector.tensor_add(y_sb[:, m, :], ps[:],
                                             xq_sb[:, m, :])
                        # LN2 stats accumulate as y tiles complete
                        if do_ln2:
                            nc.vector.tensor_copy(yb_sb[:, m, :],
                                                  y_sb[:, m, :])
                            sq = scr.tile([P, TQ], BF16, tag="sq")
                            nc.vector.tensor_mul(sq[:], yb_sb[:, m, :],
                                                 yb_sb[:, m, :])
                            nc.tensor.matmul(ps_y[:1, :], ones_col[:],
                                             yb_sb[:, m, :],
                                             start=(m == 0), stop=(m == J - 1))
                            nc.tensor.matmul(ps_yq[:1, :], ones_col[:], sq[:],
                                             start=(m == 0), stop=(m == J - 1))

                ln2_sb = sC.tile([P, J, TQ], BF16)
                if do_ln2:
                    ln_finalize(ps_y, ps_yq, yb_sb, ln2_sb, ps_mc)

                h1_sb = sC.tile([P, JF, TQ], BF16)
                for mf4 in range(JF // 4 if 'ffn' in phases else 0):
                    ws = slice(mf4 * 4 * P, (mf4 + 1) * 4 * P)
                    wt = sCw.tile([P, J, 4 * P], BF16, tag="w8f", bufs=3)
                    nc.scalar.dma_start(wt[:], w1T_r[:, :, ws])
                    for m4 in range(4):
                        mf = mf4 * 4 + m4
                        ps = ps_mc.tile([P, TQ], F32, tag="mm")
                        for j in range(J):
                            nc.tensor.matmul(
                                ps[:], wt[:, j, m4 * P:(m4 + 1) * P],
                                ln2_sb[:, j, :],
                                start=(j == 0), stop=(j == J - 1))
                        nc.scalar.activation(h1_sb[:, mf, :], ps[:], AF.Relu,
                                             bias=b1_c[:, mf:mf + 1],
                                             scale=1.0)

                for m in range(J if 'ffn' in phases else 0):
                    ms = slice(m * P, (m + 1) * P)
                    wt = sCw.tile([P, JF, P], BF16, tag="w32")
                    nc.scalar.dma_start(wt[:], w2T_r[:, :, ms])
                    ps = ps_mc.tile([P, TQ], F32, tag="mm")
                    for jf in range(JF):
                        nc.tensor.matmul(ps[:], wt[:, jf, :], h1_sb[:, jf, :],
                                         start=(jf == 0), stop=(jf == JF - 1))
                    t3 = sCw.tile([P, TQ], F32, tag="t3", bufs=2)
                    nc.vector.tensor_add(t3[:], ps[:], y_sb[:, m, :])
                    ot = sCw.tile([P, TQ], F32, tag="ot", bufs=2)
                    nc.scalar.activation(ot[:], t3[:], AF.Identity,
                                         bias=b2_c[:, m:m + 1], scale=1.0)
                    nc.sync.dma_start(outT_r[:, m, :], ot[:])

    nc.compile()
    return nc


def _get_program(S, D, H, DFF, TQ, n_iter=1, phases=FULL_PHASES):
    key = (S, D, H, DFF, TQ, n_iter, phases)
    if key not in _PROG_CACHE:
        _PROG_CACHE[key] = _build_program(S, D, H, DFF, TQ, n_iter, phases)
    return _PROG_CACHE[key]


def _run(x, mask, ln1_g, ln1_b, Wq, Wk, Wv, Wo, ln2_g, ln2_b, W1, b1, W2, b2,
         n_cores, trace=False, n_iter=1):
    B, S, D = x.shape
    DFF = W1.shape[0]
    H = D // DH
    cores_per_b = n_cores // B
    TQ = S // cores_per_b
    NQT = TQ // P

    nc = _get_program(S, D, H, DFF, TQ, n_iter)

    bf = ml_dtypes.bfloat16
    f32 = np.float32
    Wq = np.asarray(Wq, f32)
    Wk = np.asarray(Wk, f32)
    Wv = np.asarray(Wv, f32)
    Wo = np.asarray(Wo, f32)
    W1 = np.asarray(W1, f32)
    W2 = np.asarray(W2, f32)
    g1 = np.asarray(ln1_g, f32)
    b1n = np.asarray(ln1_b, f32)
    g2 = np.asarray(ln2_g, f32)
    b2n = np.asarray(ln2_b, f32)

    shared = dict(
        wqT=np.ascontiguousarray((Wq * g1).T).astype(bf),
        wkT=np.ascontiguousarray((Wk * g1).T).astype(bf),
        wvT=np.ascontiguousarray((Wv * g1).T).astype(bf),
        woT=np.ascontiguousarray(Wo.T).astype(bf),
        w1T=np.ascontiguousarray((W1 * g2).T).astype(bf),
        w2T=np.ascontiguousarray(W2.T).astype(bf),
        bq=(Wq @ b1n).astype(f32),
        bk=(Wk @ b1n).astype(f32),
        bvr=(Wv @ b1n).astype(bf),
        b1=(np.asarray(b1, f32) + W1 @ b2n).astype(f32),
        b2=np.asarray(b2, f32),
    )
    mask2d = np.asarray(mask).reshape(S, S)  # [q, k] bool
    x = np.asarray(x, f32)

    in_maps = []
    row_sets = []
    for c in range(n_cores):
        b = c // cores_per_b
        i = c % cores_per_b
        rows = np.concatenate([
            np.arange((i + cores_per_b * k) * P, (i + cores_per_b * k + 1) * P)
            for k in range(NQT)])
        row_sets.append((b, rows))
        xq = np.ascontiguousarray(x[b][rows].T)  # [D, TQ]
        mT = np.empty((S, P), f32)
        for kc in range(S // P):
            gq = i + cores_per_b * (kc // NQT)
            mT[kc * P:(kc + 1) * P, :] = \
                mask2d[gq * P:(gq + 1) * P, kc * P:(kc + 1) * P].T
        in_maps.append(dict(
            shared, xqT=xq, xqTb=xq.astype(bf),
            maskT=mT.astype(bf)))

    res = run_bass_kernel_spmd(nc, in_maps, list(range(n_cores)), trace=trace)

    out = np.empty((B, S, D), f32)
    for c in range(n_cores):
        b, rows = row_sets[c]
        out[b, rows, :] = res.results[c]["outT"].T
    return out, res


def kernel(x, mask, ln1_g, ln1_b, Wq, Wk, Wv, Wo, ln2_g, ln2_b, W1, b1, W2,
           b2):
    out, _ = _run(x, mask, ln1_g, ln1_b, Wq, Wk, Wv, Wo, ln2_g, ln2_b,
                  W1, b1, W2, b2, n_cores=8)
    return out


# revision 46
# speedup vs baseline: 1.0152x; 1.0152x over previous
"""Trainium2 Bass kernel for a pre-LN transformer decoder block.

Problem: x:[2,2048,1024] f32, causal mask, 16 heads, DFF=4096.
  out = x + Attn(LN1(x)); out = out + FFN(LN2(out))

Strategy v2 (8 NeuronCores, sequence-parallel with K/V AllGather):
  - Core c handles batch b=c//4; within the batch it OWNS 4 strided
    128-row tiles {i, i+4, i+8, i+12} (i = c%4) = 512 rows total.
  - LN1 + Q/K/V are computed only for the core's own 512 rows (no
    redundant K/V compute). K and V are then AllGather'ed within each
    batch's 4-core replica group through Internal-DRAM bounce buffers,
    giving every core the full 2048-token K/V.
  - Causal round-robin: local q-tile k (global tile i+4k) attends key
    tiles 0..4k+3 only -- a static, uniform loop across cores (62.5% of
    full attention work). The last 4 key tiles of each local q-tile get
    a data-driven mask (alive/diagonal/dead by core index) multiplied
    into exp() on the vector engine.
  - V is produced directly TRANSPOSED (keys on partitions) by swapping
    matmul operands (stationary=LN1 tile, moving=WvT), so no PE
    transposes are needed; softmax denominators come from ones columns
    interleaved in V.
  - LN gamma/beta are folded into the weights host-side; projection
    biases (W @ ln_b) are applied during PSUM evacuation.
  - Engine balance: Act does exp/relu/rstd only; evacuations go to
    DVE/Act split so exp overlaps the tensor engine.
  - Matmuls bf16 (f32 PSUM accumulate).
"""

import sys
import contextlib
import numpy as np

for _p in ("/opt/trn_rl_repo", "/root/.axon_site/_ro/trn_rl_repo"):
    if _p not in sys.path:
        sys.path.insert(0, _p)

import ml_dtypes  # noqa: E402
import concourse.bass as bass  # noqa: E402
import concourse.mybir as mybir  # noqa: E402
import concourse.tile as tile  # noqa: E402
from concourse import bacc  # noqa: E402
from concourse.bass_utils import run_bass_kernel_spmd  # noqa: E402
from concourse.hw_specs import get_activation_tables  # noqa: E402

P = 128
DH = 64
EPS = 1e-5
BF16 = mybir.dt.bfloat16
F32 = mybir.dt.float32
AF = mybir.ActivationFunctionType

_PROG_CACHE = {}


FULL_PHASES = frozenset({'lnq', 'qkv', 'cc', 'attn', 'mask', 'av', 'wo',
                         'ffn'})


def _build_program(S, D, H, DFF, TQ, n_iter=1, phases=FULL_PHASES):
    """One-core SPMD program. All cores run this identical program on
    different data; per-core causal structure lives in mask data."""
    J = D // P            # 8 feature tiles
    JF = DFF // P         # 32
    KC = S // P           # 16 key tiles
    NQT = TQ // P         # 4 local q-tiles
    G = S // TQ           # 4 cores per replica group
    VW = 2 * (DH + 1)     # v_sb cols per head-pair tile (130)
    assert TQ == 512 and H == 2 * J

    nc = bacc.Bacc(None, target_bir_lowering=False, num_devices=2 * G)

    # ---- I/O ----
    xqT = nc.dram_tensor("xqT", [D, TQ], F32, kind="ExternalInput")
    xqTb = nc.dram_tensor("xqTb", [D, TQ], BF16, kind="ExternalInput")
    maskT = nc.dram_tensor("maskT", [S, P], BF16, kind="ExternalInput")
    wqT = nc.dram_tensor("wqT", [D, D], BF16, kind="ExternalInput")
    wkT = nc.dram_tensor("wkT", [D, D], BF16, kind="ExternalInput")
    wvT = nc.dram_tensor("wvT", [D, D], BF16, kind="ExternalInput")
    woT = nc.dram_tensor("woT", [D, D], BF16, kind="ExternalInput")
    w1T = nc.dram_tensor("w1T", [D, DFF], BF16, kind="ExternalInput")
    w2T = nc.dram_tensor("w2T", [DFF, D], BF16, kind="ExternalInput")
    bq = nc.dram_tensor("bq", [D], F32, kind="ExternalInput")
    bk = nc.dram_tensor("bk", [D], F32, kind="ExternalInput")
    bvr = nc.dram_tensor("bvr", [D], BF16, kind="ExternalInput")
    b1 = nc.dram_tensor("b1", [DFF], F32, kind="ExternalInput")
    b2 = nc.dram_tensor("b2", [D], F32, kind="ExternalInput")
    outT = nc.dram_tensor("outT", [D, TQ], F32, kind="ExternalOutput")

    # ---- DRAM internals for the merged K+V collective ----
    # kvown cols: [0:J*TQ] = K by (m, t); [J*TQ:] = V^T by (kt, m, h2, d)
    KVW = 2 * J * TQ
    kvown = nc.dram_tensor("kvown", [P, KVW], BF16, kind="Internal")
    kvgat = nc.dram_tensor("kvgat", [G, P, KVW], BF16, kind="Internal")

    xqT_r = xqT.rearrange("(j p) t -> p j t", p=P)
    xqTb_r = xqTb.rearrange("(j p) t -> p j t", p=P)
    maskT_r = maskT.rearrange("(kc p) q -> p kc q", p=P)
    wqT_r = wqT.rearrange("(j p) e -> p j e", p=P)
    wkT_r = wkT.rearrange("(j p) e -> p j e", p=P)
    wvT_r = wvT.rearrange("(j p) e -> p j e", p=P)
    woT_r = woT.rearrange("(j p) e -> p j e", p=P)
    w1T_r = w1T.rearrange("(j p) f -> p j f", p=P)
    w2T_r = w2T.rearrange("(jf p) e -> p jf e", p=P)
    kvg_k = kvgat.rearrange("r p (w m t) -> r p w m t", w=2, m=J)
    kvg_v = kvgat.rearrange("r p (w kt m h d) -> r p w kt m h d",
                            w=2, kt=NQT, m=J, h=2)
    outT_r = outT.rearrange("(j p) q -> p j q", p=P)

    rgroups = [list(range(G)), list(range(G, 2 * G))]

    # NRT collectives cannot replay inside a hardware Fori loop (mesh
    # desync at iteration 2), so multi-iteration timing programs are
    # UNROLLED: straight-line sequential collectives are supported.
    # Collective-free ablations (no 'cc' phase) use a Fori loop.
    use_fori = n_iter > 1 and 'cc' not in phases and 'unroll' not in phases
    loop_cm = nc.Fori(0, n_iter) if use_fori else contextlib.nullcontext()
    with loop_cm, tile.TileContext(nc) as tc:
      for _it in range(1 if use_fori else n_iter):
        with (
            tc.tile_pool(name="const", bufs=1) as const,
            tc.tile_pool(name="persist", bufs=1) as persist,
            tc.tile_pool(name="scr", bufs=2) as scr,
            tc.tile_pool(name="scr_s", bufs=2) as scr_s,
        ):
            # Pre-load the one activation table that covers every function
            # we use (Ln/Exp/Identity/Copy/Relu); the automatic pass would
            # otherwise thrash exp_and_others <-> natural_log 5x/iter.
            if _it == 0:
                _tabs = list(get_activation_tables("gen3"))
                nc.scalar.add_instruction(mybir.InstLoadActFuncSet(
                    name=f"I-{nc.next_id()}", ins=[], outs=[],
                    act_func_set_id=_tabs.index("natural_log_exp_and_others")))

            # constants
            ones_col = const.tile([P, 1], BF16)
            nc.vector.memset(ones_col[:], 1.0)
            ones_row = const.tile([1, P], BF16)
            nc.vector.memset(ones_row[:], 1.0)
            eps_t = const.tile([1, 1], F32)
            nc.vector.memset(eps_t[:], EPS)
            bq_c = const.tile([P, J], F32)
            nc.sync.dma_start(bq_c[:], bq.rearrange("(j p) -> p j", p=P))
            bk_c = const.tile([P, J], F32)
            nc.sync.dma_start(bk_c[:], bk.rearrange("(j p) -> p j", p=P))
            bv_row = const.tile([1, D], BF16)
            nc.sync.dma_start(bv_row[:], bvr.rearrange("(o e) -> o e", o=1))
            b1_c = const.tile([P, JF], F32)
            nc.sync.dma_start(b1_c[:], b1.rearrange("(j p) -> p j", p=P))
            b2_c = const.tile([P, J], F32)
            nc.sync.dma_start(b2_c[:], b2.rearrange("(j p) -> p j", p=P))

            # persistent activations
            xq_sb = persist.tile([P, J, TQ], F32)
            nc.sync.dma_start(xq_sb[:], xqT_r)
            attn_sb = persist.tile([P, J, TQ], BF16)

            def ln_finalize(ps_x, ps_q, src_sb, out_sb, pool):
                """Given psum sums/sumsq over features: normalize src."""
                inv_d = 1.0 / D
                mu = scr_s.tile([1, TQ], F32, tag="mu", bufs=1)
                nc.scalar.mul(mu[:], ps_x[:1, :], inv_d)
                ex2 = scr_s.tile([1, TQ], F32, tag="ex2", bufs=1)
                nc.scalar.mul(ex2[:], ps_q[:1, :], inv_d)
                var = scr_s.tile([1, TQ], F32, tag="var", bufs=1)
                nc.vector.tensor_mul(var[:], mu[:], mu[:])
                nc.vector.tensor_sub(var[:], ex2[:], var[:])
                nc.scalar.activation(var[:], var[:], AF.Ln,
                                     bias=eps_t[:], scale=1.0)
                nc.scalar.activation(var[:], var[:], AF.Exp,
                                     bias=0.0, scale=-0.5)
                mub = scr_s.tile([1, TQ], BF16, tag="mub")
                nc.scalar.copy(mub[:], mu[:])
                rsb = scr_s.tile([1, TQ], BF16, tag="rsb")
                nc.scalar.copy(rsb[:], var[:])
                pmu = pool.tile([P, TQ], F32, tag="bcst")
                nc.tensor.matmul(pmu[:], ones_row[:], mub[:],
                                 start=True, stop=True)
                prs = pool.tile([P, TQ], F32, tag="bcst")
                nc.tensor.matmul(prs[:], ones_row[:], rsb[:],
                                 start=True, stop=True)
                for j in range(J):
                    t1 = scr.tile([P, TQ], F32, tag="t1")
                    nc.vector.tensor_sub(t1[:], src_sb[:, j, :], pmu[:])
                    nc.vector.tensor_mul(out_sb[:, j, :], t1[:], prs[:])

            def layer_norm(src_sb, out_sb, pool):
                """z = (x - mu) * rstd over J*P features; src/out bf16."""
                ps_x = pool.tile([P, TQ], F32, tag="stat")
                ps_q = pool.tile([P, TQ], F32, tag="stat")
                for j in range(J):
                    xb = src_sb[:, j, :]
                    sq = scr.tile([P, TQ], BF16, tag="sq")
                    nc.vector.tensor_mul(sq[:], xb, xb)
                    nc.tensor.matmul(ps_x[:1, :], ones_col[:], xb,
                                     start=(j == 0), stop=(j == J - 1))
                    nc.tensor.matmul(ps_q[:1, :], ones_col[:], sq[:],
                                     start=(j == 0), stop=(j == J - 1))
                ln_finalize(ps_x, ps_q, src_sb, out_sb, pool)

            # attention-data pool (lives through attention)
            with tc.tile_pool(name="attd", bufs=1) as attd:
                q_sb = attd.tile([P, J, TQ], BF16)
                k_sb = attd.tile([P, J, S], BF16)
                v_sb = attd.tile([P, KC, J, 2, DH + 1], BF16)
                mask_sb = attd.tile([P, KC, P], BF16)
                nc.sync.dma_start(mask_sb[:], maskT_r)

                # ---------- Phase A: LN1 + Q/K/V + collective ----------
                with tc.tile_pool(name="sA", bufs=1) as sA, \
                     tc.tile_pool(name="sAw", bufs=4) as sAw, \
                     tc.tile_pool(name="ps_mm", bufs=2, space="PSUM") as ps_mm, \
                     tc.tile_pool(name="ps_vt", bufs=1, space="PSUM") as ps_vt:
                    ln1_own = sA.tile([P, J, TQ], BF16)
                    xqb_sb = sA.tile([P, J, TQ], BF16)
                    if 'lnq' in phases:
                        nc.sync.dma_start(xqb_sb[:], xqTb_r)
                        layer_norm(xqb_sb, ln1_own, ps_mm)
                    # K projection (own rows) -> kv_stage[:, m*TQ:]
                    kv_stage = sA.tile([P, KVW], BF16)
                    for mm in range(J // 2 if 'qkv' in phases else 0):
                        ws = slice(2 * mm * P, (2 * mm + 2) * P)
                        wt = sAw.tile([P, J, 2 * P], BF16, tag="w8")
                        nc.sync.dma_start(wt[:], wkT_r[:, :, ws])
                        for m2 in range(2):
                            m = 2 * mm + m2
                            ps = ps_mm.tile([P, TQ], F32, tag="mm")
                            for j in range(J):
                                nc.tensor.matmul(
                                    ps[:], wt[:, j, m2 * P:(m2 + 1) * P],
                                    ln1_own[:, j, :],
                                    start=(j == 0), stop=(j == J - 1))
                            nc.scalar.activation(
                                kv_stage[:, m * TQ:(m + 1) * TQ], ps[:],
                                AF.Identity, bias=bk_c[:, m:m + 1], scale=1.0)
                    # V^T projection (own rows, keys on partitions) ->
                    # kv_stage[:, J*TQ + kt*D + m*P :]
                    wv_w = sA.tile([P, J, D], BF16)
                    if 'qkv' in phases:
                        nc.sync.dma_start(wv_w[:], wvT_r)
                    for kt in range(NQT if 'qkv' in phases else 0):
                        pv = ps_vt.tile([P, D], F32, tag="vt")
                        for j in range(J):
                            for eh in range(2):
                                es = slice(eh * TQ, (eh + 1) * TQ)
                                nc.tensor.matmul(
                                    pv[:, es], ln1_own[:, j, kt * P:(kt + 1) * P],
                                    wv_w[:, j, es], start=(j == 0), stop=False)
                        for eh in range(2):
                            es = slice(eh * TQ, (eh + 1) * TQ)
                            nc.tensor.matmul(pv[:, es], ones_row[:],
                                             bv_row[:, es],
                                             start=False, stop=True)
                        vb = J * TQ + kt * D
                        for m in range(J):
                            nc.vector.tensor_copy(
                                kv_stage[:, vb + m * P:vb + (m + 1) * P],
                                pv[:, m * P:(m + 1) * P])
                    if 'qkv' in phases:
                        nc.sync.dma_start(kvown[:], kv_stage[:])
                        if 'cc' in phases:
                            nc.gpsimd.collective_compute(
                                "AllGather", mybir.AluOpType.bypass,
                                replica_groups=rgroups,
                                ins=[kvown[:]], outs=[kvgat[:]])
                        # softmax-denominator ones columns
                        nc.vector.memset(v_sb[:, :, :, :, DH], 1.0)
                        # reload gathered K/V into SBUF (natural tile order)
                        for g in range(KC):
                            r, pos = g % G, g // G
                            nc.gpsimd.dma_start(
                                k_sb[:, :, g * P:(g + 1) * P],
                                kvg_k[r, :, 0, :, pos * P:(pos + 1) * P])
                            nc.gpsimd.dma_start(
                                v_sb[:, g, :, :, 0:DH], kvg_v[r, :, 1, pos])
                    # Q projection (own rows)
                    for mm in range(J // 2 if 'lnq' in phases else 0):
                        ws = slice(2 * mm * P, (2 * mm + 2) * P)
                        wt = sAw.tile([P, J, 2 * P], BF16, tag="w8")
                        nc.sync.dma_start(wt[:], wqT_r[:, :, ws])
                        for m2 in range(2):
                            m = 2 * mm + m2
                            ps = ps_mm.tile([P, TQ], F32, tag="mm")
                            for j in range(J):
                                nc.tensor.matmul(
                                    ps[:], wt[:, j, m2 * P:(m2 + 1) * P],
                                    ln1_own[:, j, :],
                                    start=(j == 0), stop=(j == J - 1))
                            nc.scalar.activation(
                                q_sb[:, m, :], ps[:], AF.Identity,
                                bias=bq_c[:, m:m + 1], scale=1.0)

                # ---------- Phase B: attention ----------
                with tc.tile_pool(name="sBe", bufs=6) as sBe, \
                     tc.tile_pool(name="ps_sc", bufs=2, space="PSUM") as ps_sc, \
                     tc.tile_pool(name="ps_zn", bufs=1, space="PSUM") as ps_zn, \
                     tc.tile_pool(name="ps_av", bufs=3, space="PSUM") as ps_av:
                    do_av_phase = 'av' in phases
                    if 'attn' in phases and not do_av_phase:
                        nc.vector.memset(attn_sb[:], 0.0)  # ablation filler
                    for m in range(J if 'attn' in phases else 0):
                        if do_av_phase:
                            pavs = [ps_av.tile([P, TQ], F32, tag="av",
                                               name=f"pav{m}_{i}")
                                    for i in range(2)]
                        pend = []

                        def do_av(kc0, e0):
                            if not do_av_phase:
                                return
                            g0 = kc0 // G
                            q0 = g0 * P
                            for h2 in range(2):
                                vs = v_sb[:, kc0, m, h2, :]
                                nc.tensor.matmul(
                                    pavs[h2][:DH + 1, q0:],
                                    vs, e0[:, h2, q0:],
                                    start=(kc0 == 0), stop=(kc0 == KC - 1),
                                    skip_group_check=True)

                        for kc in range(KC):
                            g = kc // G
                            q0 = g * P
                            ps2 = ps_sc.tile([P, 2, TQ], F32, tag="sc")
                            nc.tensor.matmul(
                                ps2[:, 0, q0:],
                                k_sb[0:DH, m, kc * P:(kc + 1) * P],
                                q_sb[0:DH, m, q0:], start=True, stop=True,
                                tile_position=(0, 0))
                            nc.tensor.matmul(
                                ps2[:, 1, q0:],
                                k_sb[DH:P, m, kc * P:(kc + 1) * P],
                                q_sb[DH:P, m, q0:], start=True, stop=True,
                                tile_position=(DH, 0))
                            ex2 = sBe.tile([P, 2, TQ], BF16, tag="ex")
                            efun = AF.Identity if 'expid' in phases else AF.Exp
                            if 'exp2' in phases:
                                for h2 in range(2):
                                    nc.scalar.activation(
                                        ex2[:, h2, q0:], ps2[:, h2, q0:],
                                        efun, scale=1.0 / np.sqrt(DH))
                            else:
                                nc.scalar.activation(ex2[:, :, q0:],
                                                     ps2[:, :, q0:], efun,
                                                     scale=1.0 / np.sqrt(DH))
                            if 'mask' in phases:
                                nc.vector.tensor_mul(
                                    ex2[:, 0, q0:q0 + P], ex2[:, 0, q0:q0 + P],
                                    mask_sb[:, kc, :])
                                nc.vector.tensor_mul(
                                    ex2[:, 1, q0:q0 + P], ex2[:, 1, q0:q0 + P],
                                    mask_sb[:, kc, :])
                            pend.append((kc, ex2))
                            while len(pend) > 3:
                                kc0, e0 = pend.pop(0)
                                do_av(kc0, e0)
                        for kc0, e0 in pend:
                            do_av(kc0, e0)
                        for h2 in range(2 if do_av_phase else 0):
                            lo = h2 * DH
                            pav = pavs[h2]
                            zr = scr_s.tile([1, TQ], F32, tag="zr")
                            nc.vector.reciprocal(zr[:], pav[DH:DH + 1, :])
                            zrb = scr_s.tile([1, TQ], BF16, tag="zrb")
                            nc.vector.tensor_copy(zrb[:], zr[:])
                            ps_z = ps_zn.tile([P, TQ], F32, tag="zn")
                            nc.tensor.matmul(ps_z[:DH, :], ones_row[:, :DH],
                                             zrb[:], start=True, stop=True)
                            zb = scr_s.tile([DH, TQ], F32, tag="zb")
                            nc.vector.tensor_copy(zb[:], ps_z[:DH, :])
                            if h2 == 0:
                                # head0 rows align with attn_sb partitions:
                                # normalize writes attn_sb directly
                                nc.vector.tensor_mul(attn_sb[0:DH, m, :],
                                                     pav[:DH, :], zb[:])
                            else:
                                stg64 = scr_s.tile([DH, TQ], BF16,
                                                   tag="stg64")
                                nc.vector.tensor_mul(stg64[:], pav[:DH, :],
                                                     zb[:])
                                nc.sync.dma_start(attn_sb[lo:lo + DH, m, :],
                                                  stg64[:])

            # ---------- Phase C: Wo + residual, LN2, FFN ----------
            with tc.tile_pool(name="sC", bufs=1) as sC, \
                 tc.tile_pool(name="sCw", bufs=3) as sCw, \
                 tc.tile_pool(name="ps_mc", bufs=2, space="PSUM") as ps_mc:
                y_sb = sC.tile([P, J, TQ], F32)
                yb_sb = sC.tile([P, J, TQ], BF16)
                do_ln2 = 'ffn' in phases and 'wo' in phases
                if do_ln2:
                    ps_y = ps_mc.tile([P, TQ], F32, tag="lny")
                    ps_yq = ps_mc.tile([P, TQ], F32, tag="lny")
                for half in range(2 if 'wo' in phases else 0):
                    ws = slice(half * 4 * P, (half + 1) * 4 * P)
                    wt = sCw.tile([P, J, 4 * P], BF16, tag="w8o", bufs=2)
                    nc.sync.dma_start(wt[:], woT_r[:, :, ws])
                    for m4 in range(4):
                        m = half * 4 + m4
                        ps = ps_mc.tile([P, TQ], F32, tag="mm")
                        for j in range(J):
                            nc.tensor.matmul(
                                ps[:], wt[:, j, m4 * P:(m4 + 1) * P],
                                attn_sb[:, j, :],
                                start=(j == 0), stop=(j == J - 1))
                        nc.vector.tensor_add(y_sb[:, m, :], ps[:],
                                             xq_sb[:, m, :])
                        # LN2 stats accumulate as y tiles complete
                        if do_ln2:
                            nc.vector.tensor_copy(yb_sb[:, m, :],
                                                  y_sb[:, m, :])
                            sq = scr.tile([P, TQ], BF16, tag="sq")
                            nc.vector.tensor_mul(sq[:], yb_sb[:, m, :],
                                                 yb_sb[:, m, :])
                            nc.tensor.matmul(ps_y[:1, :], ones_col[:],
                                             yb_sb[:, m, :],
                                             start=(m == 0), stop=(m == J - 1))
                            nc.tensor.matmul(ps_yq[:1, :], ones_col[:], sq[:],
                                             start=(m == 0), stop=(m == J - 1))

                ln2_sb = sC.tile([P, J, TQ], BF16)
                if do_ln2:
                    ln_finalize(ps_y, ps_yq, yb_sb, ln2_sb, ps_mc)

                h1_sb = sC.tile([P, JF, TQ], BF16)
                for mf4 in range(JF // 4 if 'ffn' in phases else 0):
                    ws = slice(mf4 * 4 * P, (mf4 + 1) * 4 * P)
                    wt = sCw.tile([P, J, 4 * P], BF16, tag="w8f", bufs=3)
                    nc.gpsimd.dma_start(wt[:], w1T_r[:, :, ws])
                    for m4 in range(4):
                        mf = mf4 * 4 + m4
                        ps = ps_mc.tile([P, TQ], F32, tag="mm")
                        for j in range(J):
                            nc.tensor.matmul(
                                ps[:], wt[:, j, m4 * P:(m4 + 1) * P],
                                ln2_sb[:, j, :],
                                start=(j == 0), stop=(j == J - 1))
                        nc.scalar.activation(h1_sb[:, mf, :], ps[:], AF.Relu,
                                             bias=b1_c[:, mf:mf + 1],
                                             scale=1.0)

                for m in range(J if 'ffn' in phases else 0):
                    ms = slice(m * P, (m + 1) * P)
                    wt = sCw.tile([P, JF, P], BF16, tag="w32")
                    nc.gpsimd.dma_start(wt[:], w2T_r[:, :, ms])
                    ps = ps_mc.tile([P, TQ], F32, tag="mm")
                    for jf in range(JF):
                        nc.tensor.matmul(ps[:], wt[:, jf, :], h1_sb[:, jf, :],
                                         start=(jf == 0), stop=(jf == JF - 1))
                    t3 = sCw.tile([P, TQ], F32, tag="t3", bufs=2)
                    nc.vector.tensor_add(t3[:], ps[:], y_sb[:, m, :])
                    ot = sCw.tile([P, TQ], F32, tag="ot", bufs=2)
                    nc.scalar.activation(ot[:], t3[:], AF.Identity,
                                         bias=b2_c[:, m:m + 1], scale=1.0)
                    nc.sync.dma_start(outT_r[:, m, :], ot[:])

    nc.compile()
    return nc


def _get_program(S, D, H, DFF, TQ, n_iter=1, phases=FULL_PHASES):
    key = (S, D, H, DFF, TQ, n_iter, phases)
    if key not in _PROG_CACHE:
        _PROG_CACHE[key] = _build_program(S, D, H, DFF, TQ, n_iter, phases)
    return _PROG_CACHE[key]


def _run(x, mask, ln1_g, ln1_b, Wq, Wk, Wv, Wo, ln2_g, ln2_b, W1, b1, W2, b2,
         n_cores, trace=False, n_iter=1):
    B, S, D = x.shape
    DFF = W1.shape[0]
    H = D // DH
    cores_per_b = n_cores // B
    TQ = S // cores_per_b
    NQT = TQ // P

    nc = _get_program(S, D, H, DFF, TQ, n_iter)

    bf = ml_dtypes.bfloat16
    f32 = np.float32
    Wq = np.asarray(Wq, f32)
    Wk = np.asarray(Wk, f32)
    Wv = np.asarray(Wv, f32)
    Wo = np.asarray(Wo, f32)
    W1 = np.asarray(W1, f32)
    W2 = np.asarray(W2, f32)
    g1 = np.asarray(ln1_g, f32)
    b1n = np.asarray(ln1_b, f32)
    g2 = np.asarray(ln2_g, f32)
    b2n = np.asarray(ln2_b, f32)

    shared = dict(
        wqT=np.ascontiguousarray((Wq * g1).T).astype(bf),
        wkT=np.ascontiguousarray((Wk * g1).T).astype(bf),
        wvT=np.ascontiguousarray((Wv * g1).T).astype(bf),
        woT=np.ascontiguousarray(Wo.T).astype(bf),
        w1T=np.ascontiguousarray((W1 * g2).T).astype(bf),
        w2T=np.ascontiguousarray(W2.T).astype(bf),
        bq=(Wq @ b1n).astype(f32),
        bk=(Wk @ b1n).astype(f32),
        bvr=(Wv @ b1n).astype(bf),
        b1=(np.asarray(b1, f32) + W1 @ b2n).astype(f32),
        b2=np.asarray(b2, f32),
    )
    mask2d = np.asarray(mask).reshape(S, S)  # [q, k] bool
    x = np.asarray(x, f32)

    in_maps = []
    row_sets = []
    for c in range(n_cores):
        b = c // cores_per_b
        i = c % cores_per_b
        rows = np.concatenate([
            np.arange((i + cores_per_b * k) * P, (i + cores_per_b * k + 1) * P)
            for k in range(NQT)])
        row_sets.append((b, rows))
        xq = np.ascontiguousarray(x[b][rows].T)  # [D, TQ]
        mT = np.empty((S, P), f32)
        for kc in range(S // P):
            gq = i + cores_per_b * (kc // NQT)
            mT[kc * P:(kc + 1) * P, :] = \
                mask2d[gq * P:(gq + 1) * P, kc * P:(kc + 1) * P].T
        in_maps.append(dict(
            shared, xqT=xq, xqTb=xq.astype(bf),
            maskT=mT.astype(bf)))

    res = run_bass_kernel_spmd(nc, in_maps, list(range(n_cores)), trace=trace)

    out = np.empty((B, S, D), f32)
    for c in range(n_cores):
        b, rows = row_sets[c]
        out[b, rows, :] = res.results[c]["outT"].T
    return out, res


def kernel(x, mask, ln1_g, ln1_b, Wq, Wk, Wv, Wo, ln2_g, ln2_b, W1, b1, W2,
           b2):
    out, _ = _run(x, mask, ln1_g, ln1_b, Wq, Wk, Wv, Wo, ln2_g, ln2_b,
                  W1, b1, W2, b2, n_cores=8)
    return out


# revision 47
# speedup vs baseline: 1.0846x; 1.0684x over previous
"""Trainium2 Bass kernel for a pre-LN transformer decoder block.

Problem: x:[2,2048,1024] f32, causal mask, 16 heads, DFF=4096.
  out = x + Attn(LN1(x)); out = out + FFN(LN2(out))

Strategy v2 (8 NeuronCores, sequence-parallel with K/V AllGather):
  - Core c handles batch b=c//4; within the batch it OWNS 4 strided
    128-row tiles {i, i+4, i+8, i+12} (i = c%4) = 512 rows total.
  - LN1 + Q/K/V are computed only for the core's own 512 rows (no
    redundant K/V compute). K and V are then AllGather'ed within each
    batch's 4-core replica group through Internal-DRAM bounce buffers,
    giving every core the full 2048-token K/V.
  - Causal round-robin: local q-tile k (global tile i+4k) attends key
    tiles 0..4k+3 only -- a static, uniform loop across cores (62.5% of
    full attention work). The last 4 key tiles of each local q-tile get
    a data-driven mask (alive/diagonal/dead by core index) multiplied
    into exp() on the vector engine.
  - V is produced directly TRANSPOSED (keys on partitions) by swapping
    matmul operands (stationary=LN1 tile, moving=WvT), so no PE
    transposes are needed; softmax denominators come from ones columns
    interleaved in V.
  - LN gamma/beta are folded into the weights host-side; projection
    biases (W @ ln_b) are applied during PSUM evacuation.
  - Engine balance: Act does exp/relu/rstd only; evacuations go to
    DVE/Act split so exp overlaps the tensor engine.
  - Matmuls bf16 (f32 PSUM accumulate).
"""

import sys
import contextlib
import numpy as np

for _p in ("/opt/trn_rl_repo", "/root/.axon_site/_ro/trn_rl_repo"):
    if _p not in sys.path:
        sys.path.insert(0, _p)

import ml_dtypes  # noqa: E402
import concourse.bass as bass  # noqa: E402
import concourse.mybir as mybir  # noqa: E402
import concourse.tile as tile  # noqa: E402
from concourse import bacc  # noqa: E402
from concourse.bass_utils import run_bass_kernel_spmd  # noqa: E402
from concourse.hw_specs import get_activation_tables  # noqa: E402

P = 128
DH = 64
EPS = 1e-5
BF16 = mybir.dt.bfloat16
F32 = mybir.dt.float32
AF = mybir.ActivationFunctionType

_PROG_CACHE = {}


FULL_PHASES = frozenset({'lnq', 'qkv', 'cc', 'attn', 'mask', 'av', 'wo',
                         'ffn'})


def _build_program(S, D, H, DFF, TQ, n_iter=1, phases=FULL_PHASES):
    """One-core SPMD program. All cores run this identical program on
    different data; per-core causal structure lives in mask data."""
    J = D // P            # 8 feature tiles
    JF = DFF // P         # 32
    KC = S // P           # 16 key tiles
    NQT = TQ // P         # 4 local q-tiles
    G = S // TQ           # 4 cores per replica group
    VW = 2 * (DH + 1)     # v_sb cols per head-pair tile (130)
    assert TQ == 512 and H == 2 * J

    nc = bacc.Bacc(None, target_bir_lowering=False, num_devices=2 * G)

    # ---- I/O ----
    xqT = nc.dram_tensor("xqT", [D, TQ], F32, kind="ExternalInput")
    xqTb = nc.dram_tensor("xqTb", [D, TQ], BF16, kind="ExternalInput")
    maskT = nc.dram_tensor("maskT", [S, P], BF16, kind="ExternalInput")
    wqT = nc.dram_tensor("wqT", [D, D], BF16, kind="ExternalInput")
    wkT = nc.dram_tensor("wkT", [D, D], BF16, kind="ExternalInput")
    wvT = nc.dram_tensor("wvT", [D, D], BF16, kind="ExternalInput")
    woT = nc.dram_tensor("woT", [D, D], BF16, kind="ExternalInput")
    w1T = nc.dram_tensor("w1T", [D, DFF], BF16, kind="ExternalInput")
    w2T = nc.dram_tensor("w2T", [DFF, D], BF16, kind="ExternalInput")
    bq = nc.dram_tensor("bq", [D], F32, kind="ExternalInput")
    bk = nc.dram_tensor("bk", [D], F32, kind="ExternalInput")
    bvr = nc.dram_tensor("bvr", [D], BF16, kind="ExternalInput")
    b1 = nc.dram_tensor("b1", [DFF], F32, kind="ExternalInput")
    b2 = nc.dram_tensor("b2", [D], F32, kind="ExternalInput")
    outT = nc.dram_tensor("outT", [D, TQ], F32, kind="ExternalOutput")

    # ---- DRAM internals for the merged K+V collective ----
    # kvown cols: [0:J*TQ] = K by (m, t); [J*TQ:] = V^T by (kt, m, h2, d)
    KVW = 2 * J * TQ
    kvown = nc.dram_tensor("kvown", [P, KVW], BF16, kind="Internal")
    kvgat = nc.dram_tensor("kvgat", [G, P, KVW], BF16, kind="Internal")

    xqT_r = xqT.rearrange("(j p) t -> p j t", p=P)
    xqTb_r = xqTb.rearrange("(j p) t -> p j t", p=P)
    maskT_r = maskT.rearrange("(kc p) q -> p kc q", p=P)
    wqT_r = wqT.rearrange("(j p) e -> p j e", p=P)
    wkT_r = wkT.rearrange("(j p) e -> p j e", p=P)
    wvT_r = wvT.rearrange("(j p) e -> p j e", p=P)
    woT_r = woT.rearrange("(j p) e -> p j e", p=P)
    w1T_r = w1T.rearrange("(j p) f -> p j f", p=P)
    w2T_r = w2T.rearrange("(jf p) e -> p jf e", p=P)
    kvg_k = kvgat.rearrange("r p (w m t) -> r p w m t", w=2, m=J)
    kvg_v = kvgat.rearrange("r p (w kt m h d) -> r p w kt m h d",
                            w=2, kt=NQT, m=J, h=2)
    outT_r = outT.rearrange("(j p) q -> p j q", p=P)

    rgroups = [list(range(G)), list(range(G, 2 * G))]

    # NRT collectives cannot replay inside a hardware Fori loop (mesh
    # desync at iteration 2), so multi-iteration timing programs are
    # UNROLLED: straight-line sequential collectives are supported.
    # Collective-free ablations (no 'cc' phase) use a Fori loop.
    use_fori = n_iter > 1 and 'cc' not in phases and 'unroll' not in phases
    loop_cm = nc.Fori(0, n_iter) if use_fori else contextlib.nullcontext()
    with loop_cm, tile.TileContext(nc) as tc:
      for _it in range(1 if use_fori else n_iter):
        with (
            tc.tile_pool(name="const", bufs=1) as const,
            tc.tile_pool(name="persist", bufs=1) as persist,
            tc.tile_pool(name="scr", bufs=2) as scr,
            tc.tile_pool(name="scr_s", bufs=2) as scr_s,
        ):
            # Pre-load the one activation table that covers every function
            # we use (Ln/Exp/Identity/Copy/Relu); the automatic pass would
            # otherwise thrash exp_and_others <-> natural_log 5x/iter.
            if _it == 0:
                _tabs = list(get_activation_tables("gen3"))
                nc.scalar.add_instruction(mybir.InstLoadActFuncSet(
                    name=f"I-{nc.next_id()}", ins=[], outs=[],
                    act_func_set_id=_tabs.index("natural_log_exp_and_others")))

            # constants
            ones_col = const.tile([P, 1], BF16)
            nc.vector.memset(ones_col[:], 1.0)
            ones_row = const.tile([1, P], BF16)
            nc.vector.memset(ones_row[:], 1.0)
            eps_t = const.tile([1, 1], F32)
            nc.vector.memset(eps_t[:], EPS)
            bq_c = const.tile([P, J], F32)
            nc.sync.dma_start(bq_c[:], bq.rearrange("(j p) -> p j", p=P))
            bk_c = const.tile([P, J], F32)
            nc.sync.dma_start(bk_c[:], bk.rearrange("(j p) -> p j", p=P))
            bv_row = const.tile([1, D], BF16)
            nc.sync.dma_start(bv_row[:], bvr.rearrange("(o e) -> o e", o=1))
            b1_c = const.tile([P, JF], F32)
            nc.sync.dma_start(b1_c[:], b1.rearrange("(j p) -> p j", p=P))
            b2_c = const.tile([P, J], F32)
            nc.sync.dma_start(b2_c[:], b2.rearrange("(j p) -> p j", p=P))

            # persistent activations
            xq_sb = persist.tile([P, J, TQ], F32)
            nc.sync.dma_start(xq_sb[:], xqT_r)
            attn_sb = persist.tile([P, J, TQ], BF16)

            def ln_finalize(ps_x, ps_q, src_sb, out_sb, pool):
                """Given psum sums/sumsq over features: normalize src."""
                inv_d = 1.0 / D
                mu = scr_s.tile([1, TQ], F32, tag="mu", bufs=1)
                nc.scalar.mul(mu[:], ps_x[:1, :], inv_d)
                ex2 = scr_s.tile([1, TQ], F32, tag="ex2", bufs=1)
                nc.scalar.mul(ex2[:], ps_q[:1, :], inv_d)
                var = scr_s.tile([1, TQ], F32, tag="var", bufs=1)
                nc.vector.tensor_mul(var[:], mu[:], mu[:])
                nc.vector.tensor_sub(var[:], ex2[:], var[:])
                nc.scalar.activation(var[:], var[:], AF.Ln,
                                     bias=eps_t[:], scale=1.0)
                nc.scalar.activation(var[:], var[:], AF.Exp,
                                     bias=0.0, scale=-0.5)
                mub = scr_s.tile([1, TQ], BF16, tag="mub")
                nc.scalar.copy(mub[:], mu[:])
                rsb = scr_s.tile([1, TQ], BF16, tag="rsb")
                nc.scalar.copy(rsb[:], var[:])
                pmu = pool.tile([P, TQ], F32, tag="bcst")
                nc.tensor.matmul(pmu[:], ones_row[:], mub[:],
                                 start=True, stop=True)
                prs = pool.tile([P, TQ], F32, tag="bcst")
                nc.tensor.matmul(prs[:], ones_row[:], rsb[:],
                                 start=True, stop=True)
                for j in range(J):
                    t1 = scr.tile([P, TQ], F32, tag="t1")
                    nc.vector.tensor_sub(t1[:], src_sb[:, j, :], pmu[:])
                    nc.vector.tensor_mul(out_sb[:, j, :], t1[:], prs[:])

            def layer_norm(src_sb, out_sb, pool):
                """z = (x - mu) * rstd over J*P features; src/out bf16."""
                ps_x = pool.tile([P, TQ], F32, tag="stat")
                ps_q = pool.tile([P, TQ], F32, tag="stat")
                for j in range(J):
                    xb = src_sb[:, j, :]
                    sq = scr.tile([P, TQ], BF16, tag="sq")
                    nc.vector.tensor_mul(sq[:], xb, xb)
                    nc.tensor.matmul(ps_x[:1, :], ones_col[:], xb,
                                     start=(j == 0), stop=(j == J - 1))
                    nc.tensor.matmul(ps_q[:1, :], ones_col[:], sq[:],
                                     start=(j == 0), stop=(j == J - 1))
                ln_finalize(ps_x, ps_q, src_sb, out_sb, pool)

            # attention-data pool (lives through attention)
            with tc.tile_pool(name="attd", bufs=1) as attd:
                q_sb = attd.tile([P, J, TQ], BF16)
                k_sb = attd.tile([P, J, S], BF16)
                v_sb = attd.tile([P, KC, J, 2, DH + 1], BF16)
                mask_sb = attd.tile([P, KC, P], BF16)
                nc.sync.dma_start(mask_sb[:], maskT_r)

                # ---------- Phase A: LN1 + Q/K/V + collective ----------
                with tc.tile_pool(name="sA", bufs=1) as sA, \
                     tc.tile_pool(name="sAw", bufs=4) as sAw, \
                     tc.tile_pool(name="ps_mm", bufs=2, space="PSUM") as ps_mm, \
                     tc.tile_pool(name="ps_vt", bufs=1, space="PSUM") as ps_vt:
                    ln1_own = sA.tile([P, J, TQ], BF16)
                    xqb_sb = sA.tile([P, J, TQ], BF16)
                    if 'lnq' in phases:
                        nc.sync.dma_start(xqb_sb[:], xqTb_r)
                        layer_norm(xqb_sb, ln1_own, ps_mm)
                    # K projection (own rows) -> kv_stage[:, m*TQ:]
                    kv_stage = sA.tile([P, KVW], BF16)
                    for mm in range(J // 2 if 'qkv' in phases else 0):
                        ws = slice(2 * mm * P, (2 * mm + 2) * P)
                        wt = sAw.tile([P, J, 2 * P], BF16, tag="w8")
                        nc.sync.dma_start(wt[:], wkT_r[:, :, ws])
                        for m2 in range(2):
                            m = 2 * mm + m2
                            ps = ps_mm.tile([P, TQ], F32, tag="mm")
                            for j in range(J):
                                nc.tensor.matmul(
                                    ps[:], wt[:, j, m2 * P:(m2 + 1) * P],
                                    ln1_own[:, j, :],
                                    start=(j == 0), stop=(j == J - 1))
                            nc.scalar.activation(
                                kv_stage[:, m * TQ:(m + 1) * TQ], ps[:],
                                AF.Identity, bias=bk_c[:, m:m + 1], scale=1.0)
                    # V^T projection (own rows, keys on partitions) ->
                    # kv_stage[:, J*TQ + kt*D + m*P :]
                    wv_w = sA.tile([P, J, D], BF16)
                    if 'qkv' in phases:
                        nc.sync.dma_start(wv_w[:], wvT_r)
                    for kt in range(NQT if 'qkv' in phases else 0):
                        pv = ps_vt.tile([P, D], F32, tag="vt")
                        for j in range(J):
                            for eh in range(2):
                                es = slice(eh * TQ, (eh + 1) * TQ)
                                nc.tensor.matmul(
                                    pv[:, es], ln1_own[:, j, kt * P:(kt + 1) * P],
                                    wv_w[:, j, es], start=(j == 0), stop=False)
                        for eh in range(2):
                            es = slice(eh * TQ, (eh + 1) * TQ)
                            nc.tensor.matmul(pv[:, es], ones_row[:],
                                             bv_row[:, es],
                                             start=False, stop=True)
                        vb = J * TQ + kt * D
                        for m in range(J):
                            nc.vector.tensor_copy(
                                kv_stage[:, vb + m * P:vb + (m + 1) * P],
                                pv[:, m * P:(m + 1) * P])
                    if 'qkv' in phases:
                        nc.sync.dma_start(kvown[:], kv_stage[:])
                        if 'cc' in phases:
                            nc.gpsimd.collective_compute(
                                "AllGather", mybir.AluOpType.bypass,
                                replica_groups=rgroups,
                                ins=[kvown[:]], outs=[kvgat[:]])
                        # softmax-denominator ones columns
                        nc.vector.memset(v_sb[:, :, :, :, DH], 1.0)
                        # reload gathered K/V into SBUF (natural tile order)
                        for g in range(KC):
                            r, pos = g % G, g // G
                            nc.gpsimd.dma_start(
                                k_sb[:, :, g * P:(g + 1) * P],
                                kvg_k[r, :, 0, :, pos * P:(pos + 1) * P])
                            nc.gpsimd.dma_start(
                                v_sb[:, g, :, :, 0:DH], kvg_v[r, :, 1, pos])
                    # Q projection (own rows)
                    for mm in range(J // 2 if 'lnq' in phases else 0):
                        ws = slice(2 * mm * P, (2 * mm + 2) * P)
                        wt = sAw.tile([P, J, 2 * P], BF16, tag="w8")
                        nc.sync.dma_start(wt[:], wqT_r[:, :, ws])
                        for m2 in range(2):
                            m = 2 * mm + m2
                            ps = ps_mm.tile([P, TQ], F32, tag="mm")
                            for j in range(J):
                                nc.tensor.matmul(
                                    ps[:], wt[:, j, m2 * P:(m2 + 1) * P],
                                    ln1_own[:, j, :],
                                    start=(j == 0), stop=(j == J - 1))
                            nc.scalar.activation(
                                q_sb[:, m, :], ps[:], AF.Identity,
                                bias=bq_c[:, m:m + 1], scale=1.0)

                # ---------- Phase B: attention ----------
                with tc.tile_pool(name="sBe", bufs=6) as sBe, \
                     tc.tile_pool(name="ps_sc", bufs=2, space="PSUM") as ps_sc, \
                     tc.tile_pool(name="ps_zn", bufs=1, space="PSUM") as ps_zn, \
                     tc.tile_pool(name="ps_av", bufs=3, space="PSUM") as ps_av:
                    do_av_phase = 'av' in phases
                    if 'attn' in phases and not do_av_phase:
                        nc.vector.memset(attn_sb[:], 0.0)  # ablation filler
                    for m in range(J if 'attn' in phases else 0):
                        if do_av_phase:
                            pavs = [ps_av.tile([P, TQ], F32, tag="av",
                                               name=f"pav{m}_{i}")
                                    for i in range(2)]
                        pend = []

                        def do_av(kc0, e0):
                            if not do_av_phase:
                                return
                            g0 = kc0 // G
                            q0 = g0 * P
                            for h2 in range(2):
                                vs = v_sb[:, kc0, m, h2, :]
                                nc.tensor.matmul(
                                    pavs[h2][:DH + 1, q0:],
                                    vs, e0[:, h2, q0:],
                                    start=(kc0 == 0), stop=(kc0 == KC - 1),
                                    skip_group_check=True)

                        for kc in range(KC):
                            g = kc // G
                            q0 = g * P
                            ps2 = ps_sc.tile([P, 2, TQ], F32, tag="sc")
                            nc.tensor.matmul(
                                ps2[:, 0, q0:],
                                k_sb[0:DH, m, kc * P:(kc + 1) * P],
                                q_sb[0:DH, m, q0:], start=True, stop=True,
                                tile_position=(0, 0))
                            nc.tensor.matmul(
                                ps2[:, 1, q0:],
                                k_sb[DH:P, m, kc * P:(kc + 1) * P],
                                q_sb[DH:P, m, q0:], start=True, stop=True,
                                tile_position=(DH, 0))
                            ex2 = sBe.tile([P, 2, TQ], BF16, tag="ex")
                            efun = AF.Identity if 'expid' in phases else AF.Exp
                            if 'exp2' in phases:
                                for h2 in range(2):
                                    nc.scalar.activation(
                                        ex2[:, h2, q0:], ps2[:, h2, q0:],
                                        efun, scale=1.0 / np.sqrt(DH))
                            else:
                                nc.scalar.activation(ex2[:, :, q0:],
                                                     ps2[:, :, q0:], efun,
                                                     scale=1.0 / np.sqrt(DH))
                            if 'mask' in phases:
                                nc.vector.tensor_mul(
                                    ex2[:, 0, q0:q0 + P], ex2[:, 0, q0:q0 + P],
                                    mask_sb[:, kc, :])
                                nc.vector.tensor_mul(
                                    ex2[:, 1, q0:q0 + P], ex2[:, 1, q0:q0 + P],
                                    mask_sb[:, kc, :])
                            pend.append((kc, ex2))
                            while len(pend) > 3:
                                kc0, e0 = pend.pop(0)
                                do_av(kc0, e0)
                        for kc0, e0 in pend:
                            do_av(kc0, e0)
                        for h2 in range(2 if do_av_phase else 0):
                            lo = h2 * DH
                            pav = pavs[h2]
                            zr = scr_s.tile([1, TQ], F32, tag="zr")
                            nc.vector.reciprocal(zr[:], pav[DH:DH + 1, :])
                            zrb = scr_s.tile([1, TQ], BF16, tag="zrb")
                            nc.vector.tensor_copy(zrb[:], zr[:])
                            ps_z = ps_zn.tile([P, TQ], F32, tag="zn")
                            nc.tensor.matmul(ps_z[:DH, :], ones_row[:, :DH],
                                             zrb[:], start=True, stop=True)
                            zb = scr_s.tile([DH, TQ], F32, tag="zb")
                            nc.vector.tensor_copy(zb[:], ps_z[:DH, :])
                            if h2 == 0:
                                # head0 rows align with attn_sb partitions:
                                # normalize writes attn_sb directly
                                nc.vector.tensor_mul(attn_sb[0:DH, m, :],
                                                     pav[:DH, :], zb[:])
                            else:
                                stg64 = scr_s.tile([DH, TQ], BF16,
                                                   tag="stg64")
                                nc.vector.tensor_mul(stg64[:], pav[:DH, :],
                                                     zb[:])
                                nc.sync.dma_start(attn_sb[lo:lo + DH, m, :],
                                                  stg64[:])

            # ---------- Phase C: Wo + residual, LN2, FFN ----------
            with tc.tile_pool(name="sC", bufs=1) as sC, \
                 tc.tile_pool(name="sCw", bufs=3) as sCw, \
                 tc.tile_pool(name="ps_mc", bufs=2, space="PSUM") as ps_mc:
                y_sb = sC.tile([P, J, TQ], F32)
                yb_sb = sC.tile([P, J, TQ], BF16)
                do_ln2 = 'ffn' in phases and 'wo' in phases
                if do_ln2:
                    ps_y = ps_mc.tile([P, TQ], F32, tag="lny")
                    ps_yq = ps_mc.tile([P, TQ], F32, tag="lny")
                for half in range(2 if 'wo' in phases else 0):
                    ws = slice(half * 4 * P, (half + 1) * 4 * P)
                    wt = sCw.tile([P, J, 4 * P], BF16, tag="w8o", bufs=2)
                    nc.sync.dma_start(wt[:], woT_r[:, :, ws])
                    for m4 in range(4):
                        m = half * 4 + m4
                        ps = ps_mc.tile([P, TQ], F32, tag="mm")
                        for j in range(J):
                            nc.tensor.matmul(
                                ps[:], wt[:, j, m4 * P:(m4 + 1) * P],
                                attn_sb[:, j, :],
                                start=(j == 0), stop=(j == J - 1))
                        nc.vector.tensor_add(y_sb[:, m, :], ps[:],
                                             xq_sb[:, m, :])
                        # LN2 stats accumulate as y tiles complete
                        if do_ln2:
                            nc.vector.tensor_copy(yb_sb[:, m, :],
                                                  y_sb[:, m, :])
                            sq = scr.tile([P, TQ], BF16, tag="sq")
                            nc.vector.tensor_mul(sq[:], yb_sb[:, m, :],
                                                 yb_sb[:, m, :])
                            nc.tensor.matmul(ps_y[:1, :], ones_col[:],
                                             yb_sb[:, m, :],
                                             start=(m == 0), stop=(m == J - 1))
                            nc.tensor.matmul(ps_yq[:1, :], ones_col[:], sq[:],
                                             start=(m == 0), stop=(m == J - 1))

                ln2_sb = sC.tile([P, J, TQ], BF16)
                if do_ln2:
                    ln_finalize(ps_y, ps_yq, yb_sb, ln2_sb, ps_mc)

                h1_sb = sC.tile([P, JF, TQ], BF16)
                for mf4 in range(JF // 4 if 'ffn' in phases else 0):
                    ws = slice(mf4 * 4 * P, (mf4 + 1) * 4 * P)
                    wt = sCw.tile([P, J, 4 * P], BF16, tag="w8f", bufs=3)
                    nc.sync.dma_start(wt[:], w1T_r[:, :, ws])
                    for m4 in range(4):
                        mf = mf4 * 4 + m4
                        ps = ps_mc.tile([P, TQ], F32, tag="mm")
                        for j in range(J):
                            nc.tensor.matmul(
                                ps[:], wt[:, j, m4 * P:(m4 + 1) * P],
                                ln2_sb[:, j, :],
                                start=(j == 0), stop=(j == J - 1))
                        nc.scalar.activation(h1_sb[:, mf, :], ps[:], AF.Relu,
                                             bias=b1_c[:, mf:mf + 1],
                                             scale=1.0)

                for m in range(J if 'ffn' in phases else 0):
                    ms = slice(m * P, (m + 1) * P)
                    wt = sCw.tile([P, JF, P], BF16, tag="w32")
                    nc.sync.dma_start(wt[:], w2T_r[:, :, ms])
                    ps = ps_mc.tile([P, TQ], F32, tag="mm")
                    for jf in range(JF):
                        nc.tensor.matmul(ps[:], wt[:, jf, :], h1_sb[:, jf, :],
                                         start=(jf == 0), stop=(jf == JF - 1))
                    t3 = sCw.tile([P, TQ], F32, tag="t3", bufs=2)
                    nc.vector.tensor_add(t3[:], ps[:], y_sb[:, m, :])
                    ot = sCw.tile([P, TQ], F32, tag="ot", bufs=2)
                    nc.scalar.activation(ot[:], t3[:], AF.Identity,
                                         bias=b2_c[:, m:m + 1], scale=1.0)
                    nc.sync.dma_start(outT_r[:, m, :], ot[:])

    nc.compile()
    return nc


def _get_program(S, D, H, DFF, TQ, n_iter=1, phases=FULL_PHASES):
    key = (S, D, H, DFF, TQ, n_iter, phases)
    if key not in _PROG_CACHE:
        _PROG_CACHE[key] = _build_program(S, D, H, DFF, TQ, n_iter, phases)
    return _PROG_CACHE[key]


def _run(x, mask, ln1_g, ln1_b, Wq, Wk, Wv, Wo, ln2_g, ln2_b, W1, b1, W2, b2,
         n_cores, trace=False, n_iter=1):
    B, S, D = x.shape
    DFF = W1.shape[0]
    H = D // DH
    cores_per_b = n_cores // B
    TQ = S // cores_per_b
    NQT = TQ // P

    nc = _get_program(S, D, H, DFF, TQ, n_iter)

    bf = ml_dtypes.bfloat16
    f32 = np.float32
    Wq = np.asarray(Wq, f32)
    Wk = np.asarray(Wk, f32)
    Wv = np.asarray(Wv, f32)
    Wo = np.asarray(Wo, f32)
    W1 = np.asarray(W1, f32)
    W2 = np.asarray(W2, f32)
    g1 = np.asarray(ln1_g, f32)
    b1n = np.asarray(ln1_b, f32)
    g2 = np.asarray(ln2_g, f32)
    b2n = np.asarray(ln2_b, f32)

    shared = dict(
        wqT=np.ascontiguousarray((Wq * g1).T).astype(bf),
        wkT=np.ascontiguousarray((Wk * g1).T).astype(bf),
        wvT=np.ascontiguousarray((Wv * g1).T).astype(bf),
        woT=np.ascontiguousarray(Wo.T).astype(bf),
        w1T=np.ascontiguousarray((W1 * g2).T).astype(bf),
        w2T=np.ascontiguousarray(W2.T).astype(bf),
        bq=(Wq @ b1n).astype(f32),
        bk=(Wk @ b1n).astype(f32),
        bvr=(Wv @ b1n).astype(bf),
        b1=(np.asarray(b1, f32) + W1 @ b2n).astype(f32),
        b2=np.asarray(b2, f32),
    )
    mask2d = np.asarray(mask).reshape(S, S)  # [q, k] bool
    x = np.asarray(x, f32)

    in_maps = []
    row_sets = []
    for c in range(n_cores):
        b = c // cores_per_b
        i = c % cores_per_b
        rows = np.concatenate([
            np.arange((i + cores_per_b * k) * P, (i + cores_per_b * k + 1) * P)
            for k in range(NQT)])
        row_sets.append((b, rows))
        xq = np.ascontiguousarray(x[b][rows].T)  # [D, TQ]
        mT = np.empty((S, P), f32)
        for kc in range(S // P):
            gq = i + cores_per_b * (kc // NQT)
            mT[kc * P:(kc + 1) * P, :] = \
                mask2d[gq * P:(gq + 1) * P, kc * P:(kc + 1) * P].T
        in_maps.append(dict(
            shared, xqT=xq, xqTb=xq.astype(bf),
            maskT=mT.astype(bf)))

    res = run_bass_kernel_spmd(nc, in_maps, list(range(n_cores)), trace=trace)

    out = np.empty((B, S, D), f32)
    for c in range(n_cores):
        b, rows = row_sets[c]
        out[b, rows, :] = res.results[c]["outT"].T
    return out, res


def kernel(x, mask, ln1_g, ln1_b, Wq, Wk, Wv, Wo, ln2_g, ln2_b, W1, b1, W2,
           b2):
    out, _ = _run(x, mask, ln1_g, ln1_b, Wq, Wk, Wv, Wo, ln2_g, ln2_b,
                  W1, b1, W2, b2, n_cores=8)
    return out
